# revision 9
# baseline (speedup 1.0000x reference)
"""FactorMask v2: multi-engine rebalance (P-route).

Math per output element (per channel c, 9 taps e):
  t_e = m_e*x_sh - k_e ;  A_e = |t_e| ;  S = sum_e m_e*x_sh  (PE diag)
  m' = S/9 - kbar      ;  n1 = sum A_e ;  P_e = max(A_e, m')
  var = 2*sum(P) - n1 - 9*m'
  y = (n1-9)(var-9)/81 = (u2-2)*w,  u2 = (n1+9)/9,  w = (2/9)(SumP-4.5m') - u2

Engines: PE does the three plain sums (S via diag(m_e) weights, n1 and
SumP via identity weights, m'-fold via diag(-4.5)); Act computes A-taps
(Abs activation) + PSUM evicts; DVE computes A-taps (ts pairs at 4x) and
P-taps (tt max at 2x) + optional SumP trees; Pool computes P-taps and the
two fin ops (stt).  Layout: partition p = 32*quarter + channel; all taps
are free-dim AP offsets on a padded [128, 58, 226] fp16 input.
"""

import os
import sys

for _p in ("/opt/trn_rl_repo", "/opt/pypackages"):
    if _p not in sys.path:
        sys.path.insert(0, _p)

import numpy as np

import concourse.bacc as bacc
import concourse.mybir as mybir
import concourse.tile as tile
from concourse.bass_utils import run_bass_kernel_spmd

B, H, W, C = 8, 224, 224, 32
E = 9
NCORES = 8
Q = 4
RQ = H // Q          # 56
RA = RQ + 2          # 58
WP = W + 2           # 226
P = 128

F32 = mybir.dt.float32
F16 = mybir.dt.float16
F8 = mybir.dt.float8e4

TAPS = [(0, 0)] + [
    (dy, dx) for dy in (-1, 0, 1) for dx in (-1, 0, 1) if not (dy == 0 and dx == 0)
]

# ---- tunables ----
RB = int(os.environ.get("FM_RB", "8"))          # band rows
# taps 0..N_ACT-1: Act produces true A (Abs); rest: DVE 1-op A' = |x - c_e|
# (m_e folded into the SumA matmul weight and Pool's scaled-P stt)
N_ACT = int(os.environ.get("FM_NACT", "7"))
# among Act taps, how many have P on Pool (A'-taps' P is always Pool)
N_PACT = int(os.environ.get("FM_NPACT", "0"))
# number of bands whose SumP runs as DVE tt-tree instead of PE
NP_DVE = int(os.environ.get("FM_NPDVE", "4"))
TREETAIL = bool(int(os.environ.get("FM_TREETAIL", "0")))
# fp8 stacks + DoubleRow paired sum-matmuls for SumA/SumP
FP8 = bool(int(os.environ.get("FM_FP8", "0")))
# walrus rejects gpsimd elementwise ops: route P-taps + fins off Pool
NOPOOL = bool(int(os.environ.get("FM_NOPOOL", "1")))
SDT = F8 if FP8 else F16
SEG = 2                                          # rows per matmul psum slice

_CACHE = {}

# Optionally re-enable walrus LDWEIGHTS dedup (pipeline default disables it;
# our identity-sum matmul runs reuse the same stationary weights 9x).
if bool(int(os.environ.get("FM_LDWOPT", "0"))):
    import concourse.bass_utils as _bu

    _orig_run_command = _bu.run_command

    def _run_command_ldwopt(argv, **kw):
        argv = [
            "--enable-ldw-opt=true" if a == "--enable-ldw-opt=false" else a
            for a in argv
        ]
        return _orig_run_command(argv, **kw)

    _bu.run_command = _run_command_ldwopt


def _build_program(num_devices=NCORES):
    nc = bacc.Bacc(
        "TRN2", target_bir_lowering=False, debug=False, num_devices=num_devices
    )
    x_d = nc.dram_tensor("x", [P, RA, WP], F16, kind="ExternalInput").ap()
    pv_d = nc.dram_tensor("pv", [P, 30], F32, kind="ExternalInput").ap()
    wm_d = nc.dram_tensor("wm", [P, E + 4, P], F16, kind="ExternalInput").ap()
    wa_d = nc.dram_tensor("wa", [P, 12, P], F8, kind="ExternalInput").ap()
    y_d = nc.dram_tensor("y", [P, RQ, W], F16, kind="ExternalOutput").ap()

    with tile.TileContext(nc) as tc:
        _emit(tc, nc, x_d, pv_d, wm_d, wa_d, y_d)
    nc.compile()
    return nc


def _emit(tc, nc, x_d, pv_d, wm_d, wa_d, y_d):
    assert RQ % RB == 0 and RB % SEG == 0
    Abs = mybir.ActivationFunctionType.Abs
    Ident = mybir.ActivationFunctionType.Identity
    add = mybir.AluOpType.add
    sub = mybir.AluOpType.subtract
    mult = mybir.AluOpType.mult
    vmax = mybir.AluOpType.max
    band_ = mybir.AluOpType.bitwise_and
    U16 = mybir.dt.uint16

    sizes = [int(s) for s in os.environ.get("FM_BANDS", "2,4,8,8,8,8,8,8,2").split(",")]
    assert sum(sizes) == RQ and all(s % SEG == 0 for s in sizes)
    starts = [sum(sizes[:i]) for i in range(len(sizes))]
    nbands = len(sizes)
    act_taps = set(range(N_ACT))

    with (
        tc.tile_pool(name="const", bufs=1) as cpool,
        tc.tile_pool(name="xin", bufs=3) as xpool,
        tc.tile_pool(name="stk", bufs=int(os.environ.get("FM_SBUFS", "2"))) as spool,
        tc.tile_pool(name="wrk", bufs=2) as wpool,
        tc.tile_pool(name="acc", bufs=1, space="PSUM") as ppool,
    ):
        pv = cpool.tile([P, 30], F32)
        nc.sync.dma_start(pv[:], pv_d[:])
        wm = cpool.tile([P, E + 4, P], F16)
        nc.sync.dma_start(wm[:], wm_d[:])

        st = {}

        def front(band):
            r0 = starts[band]
            rb = sizes[band]
            xb = xpool.tile([P, rb + 2, WP], F16, tag="xb")
            nc.sync.dma_start(xb[:], x_d[:, r0 : r0 + rb + 2, :])

            def shift(dy, dx, i0, n):
                return xb[
                    :, 1 + dy + i0 : 1 + dy + i0 + n, 1 + dx : 1 + dx + W
                ]

            # S (PE): 9 diag(m_e) matmuls per SEG slice -> m' (Act evict)
            mp = wpool.tile([P, rb, W], F16, tag="mp")
            for s in range(rb // SEG):
                i0 = s * SEG
                sp = ppool.tile([P, SEG * W], F32, tag="sp", bufs=2)
                for e, (dy, dx) in enumerate(TAPS):
                    nc.tensor.matmul(
                        sp[:],
                        wm[:, e, :],
                        shift(dy, dx, i0, SEG),
                        start=(e == 0),
                        stop=(e == E - 1),
                    )
                nc.scalar.activation(
                    mp[:, i0 : i0 + SEG, :],
                    sp[:],
                    Ident,
                    bias=pv[:, 18:19],
                    scale=1.0 / E,
                )

            # A-taps: true A = |m x - k| via Act (1 op) or DVE (2 ts ops)
            A = spool.tile([P, E, rb, W], F16, tag="A")
            for e, (dy, dx) in enumerate(TAPS):
                if e in act_taps:
                    nc.scalar.activation(
                        A[:, e],
                        shift(dy, dx, 0, rb),
                        Abs,
                        bias=pv[:, E + e : E + e + 1],
                        scale=pv[:, e : e + 1],
                    )
                else:
                    tq = wpool.tile([P, rb, W], F16, tag="tq")
                    nc.vector.tensor_scalar(
                        tq[:],
                        shift(dy, dx, 0, rb),
                        pv[:, e : e + 1],
                        pv[:, E + e : E + e + 1],
                        mult,
                        add,
                    )
                    nc.vector.tensor_scalar(
                        A[:, e].bitcast(U16), tq[:].bitcast(U16),
                        0x7FFF, None, band_,
                    )
            st[band] = (mp, A)

        def back(band):
            r0 = starts[band]
            rb = sizes[band]
            mp, A = st.pop(band)

            # P-taps: P_e = max(A_e, m')  (DVE tt)
            Pt = spool.tile([P, E, rb, W], F16, tag="P", bufs=2)
            for e in range(E):
                nc.vector.tensor_tensor(Pt[:, e], A[:, e], mp[:], vmax)

            # n1 = sum A (PE identity) -> u2 = (n1+9)/9 (Act evict)
            u2 = wpool.tile([P, rb, W], F16, tag="u2")
            for s in range(rb // SEG):
                i0 = s * SEG
                np_ = ppool.tile([P, SEG * W], F32, tag="np", bufs=3)
                for e in range(E):
                    nc.tensor.matmul(
                        np_[:],
                        wm[:, E, :],
                        A[:, e, i0 : i0 + SEG, :],
                        start=(e == 0),
                        stop=(e == E - 1),
                    )
                nc.scalar.activation(
                    u2[:, i0 : i0 + SEG, :],
                    np_[:],
                    Ident,
                    bias=1.0,
                    scale=1.0 / E,
                )

            # Pp = (2/9)(sum P) - m'  -> q = Pp - u2 = (var-9)/9
            q_t = wpool.tile([P, rb, W], F16, tag="q")
            if (band < nbands - NP_DVE) if TREETAIL else (band >= NP_DVE):
                for s in range(rb // SEG):
                    i0 = s * SEG
                    pp = ppool.tile([P, SEG * W], F32, tag="pp", bufs=3)
                    for e in range(E):
                        nc.tensor.matmul(
                            pp[:],
                            wm[:, E + 2, :],
                            Pt[:, e, i0 : i0 + SEG, :],
                            start=(e == 0),
                            stop=False,
                        )
                    nc.tensor.matmul(
                        pp[:],
                        wm[:, E + 1, :],
                        mp[:, i0 : i0 + SEG, :],
                        start=False,
                        stop=True,
                    )
                    nc.vector.tensor_tensor(
                        q_t[:, i0 : i0 + SEG, :],
                        pp[:],
                        u2[:, i0 : i0 + SEG, :],
                        sub,
                    )
            else:
                acc = wpool.tile([P, rb, W], F16, tag="acc")
                acc2 = wpool.tile([P, rb, W], F16, tag="acc2")
                nc.vector.tensor_scalar(acc[:], mp[:], -4.5, None, mult)
                cur, other = acc, acc2
                for e in range(E):
                    nc.vector.tensor_tensor(other[:], cur[:], Pt[:, e], add)
                    cur, other = other, cur
                nc.vector.tensor_scalar(cur[:], cur[:], 2.0 / E, None, mult)
                nc.vector.tensor_tensor(q_t[:], cur[:], u2[:], sub)

            # u0 = u2 - 2 ; y = u0 * q
            u0 = wpool.tile([P, rb, W], F16, tag="u0")
            nc.vector.tensor_scalar(u0[:], u2[:], 2.0, None, sub)
            yb = wpool.tile([P, rb, W], F16, tag="yb")
            nc.vector.tensor_tensor(yb[:], u0[:], q_t[:], mult)
            nc.sync.dma_start(y_d[:, r0 : r0 + rb, :], yb[:])

        depth = int(os.environ.get("FM_DEPTH", "1"))
        for band in range(nbands + depth):
            if band < nbands:
                front(band)
            if band >= depth:
                back(band - depth)


def _host_pack(inp, kern, mask):
    inp = np.asarray(inp, dtype=np.float32)
    kern = np.asarray(kern, dtype=np.float32).reshape(E, C)
    mask = np.asarray(mask, dtype=np.float32).reshape(E, C)

    m = np.abs(mask) / (np.abs(mask).max() + np.float32(1e-6))  # [E,C]
    kbar = kern.mean(axis=0)

    cidx = np.arange(P) % C
    pv = np.zeros((P, 30), np.float32)
    for e in range(E):
        pv[:, e] = m[e][cidx]
        pv[:, E + e] = -kern[e][cidx]
        pv[:, 20 + e] = (kern[e] / m[e])[cidx]
    pv[:, 18] = -kbar[cidx]
    pv[:, 19] = -1.0
    pv[:, 29] = -2.0

    wm = np.zeros((P, E + 4, P), np.float16)
    rng = np.arange(P)
    for e in range(E):
        wm[rng, e, rng] = m[e][cidx]
    wm[rng, E, rng] = 1.0
    wm[rng, E + 1, rng] = -1.0          # fold: -1 * m'  (scaled SumP)
    wm[rng, E + 2, rng] = 2.0 / E       # (2/9)*I for SumP
    wm[rng, E + 3, rng] = -2.0          # -2 * ones

    import ml_dtypes

    wa = np.zeros((P, 12, P), ml_dtypes.float8_e4m3)
    for e in range(11):
        wa[rng, e, rng] = 1.0

    in_maps = []
    for b in range(B):
        padded = np.pad(inp[b], ((1, 1), (1, 1), (0, 0)))  # [226,226,32]
        qs = np.stack(
            [padded[RQ * q : RQ * q + RA] for q in range(Q)], axis=0
        )
        x_dev = np.ascontiguousarray(
            qs.transpose(0, 3, 1, 2).reshape(P, RA, WP).astype(np.float16)
        )
        in_maps.append({"x": x_dev, "pv": pv, "wm": wm, "wa": wa})
    return in_maps


def _host_unpack(results):
    out = np.empty((B, H, W, C), np.float32)
    for b in range(B):
        y = np.asarray(results[b]["y"], dtype=np.float32).reshape(Q, C, RQ, W)
        out[b] = y.transpose(0, 2, 3, 1).reshape(H, W, C)
    return out


LAST_PROFILE = {}


def _install_ntff_shim():
    import contextlib
    import ctypes
    import types

    if "antenv.axon_hooks" in sys.modules:
        return
    so_path = "/opt/axon/libaxon_pjrt.so"
    try:
        lib = ctypes.CDLL(so_path)
    except OSError:
        return
    if not hasattr(lib, "axon_start_nrt_profile"):
        return
    lib.axon_start_nrt_profile.argtypes = [
        ctypes.POINTER(ctypes.c_int64),
        ctypes.c_size_t,
    ]
    lib.axon_start_nrt_profile.restype = ctypes.c_int64
    lib.axon_stop_nrt_profile.argtypes = [ctypes.c_char_p]
    lib.axon_stop_nrt_profile.restype = ctypes.c_int64

    @contextlib.contextmanager
    def _hook(output_dir, device_ids):
        import jax

        jax.devices()
        if device_ids:
            ids = (ctypes.c_int64 * len(device_ids))(*device_ids)
            rc = lib.axon_start_nrt_profile(ids, len(device_ids))
        else:
            rc = lib.axon_start_nrt_profile(None, 0)
        if rc != 0:
            raise RuntimeError(f"axon_start_nrt_profile rc={rc}")
        try:
            yield
        finally:
            n = lib.axon_stop_nrt_profile(str(output_dir).encode())
            if n < 0:
                raise RuntimeError(f"axon_stop_nrt_profile rc={n}")
            print(f"ntff profile: {n} file(s) written to {output_dir}")

    mod = types.ModuleType("antenv.axon_hooks")
    mod._hook = _hook
    mod.get_axon_ntff_profile_hook = lambda: mod._hook
    mod.set_axon_ntff_profile_hook = lambda h: setattr(mod, "_hook", h)
    sys.modules["antenv.axon_hooks"] = mod


def kernel(inp, kernel, mask):
    if "nc" not in _CACHE:
        _CACHE["nc"] = _build_program()
    nc = _CACHE["nc"]

    in_maps = _host_pack(inp, kernel, mask)
    trace = bool(int(os.environ.get("FM_TRACE", "0")))
    if trace:
        _install_ntff_shim()
    res = run_bass_kernel_spmd(
        nc, in_maps, core_ids=list(range(NCORES)), trace=trace
    )
    LAST_PROFILE["exec_time_ns"] = res.exec_time_ns
    LAST_PROFILE["mean_exec_time_ns"] = res.mean_exec_time_ns
    return _host_unpack(res.results)


# revision 14
# speedup vs baseline: 1.2036x; 1.2036x over previous
"""FactorMask v2: multi-engine rebalance (P-route).

Math per output element (per channel c, 9 taps e):
  t_e = m_e*x_sh - k_e ;  A_e = |t_e| ;  S = sum_e m_e*x_sh  (PE diag)
  m' = S/9 - kbar      ;  n1 = sum A_e ;  P_e = max(A_e, m')
  var = 2*sum(P) - n1 - 9*m'
  y = (n1-9)(var-9)/81 = (u2-2)*w,  u2 = (n1+9)/9,  w = (2/9)(SumP-4.5m') - u2

Engines: PE does the three plain sums (S via diag(m_e) weights, n1 and
SumP via identity weights, m'-fold via diag(-4.5)); Act computes A-taps
(Abs activation) + PSUM evicts; DVE computes A-taps (ts pairs at 4x) and
P-taps (tt max at 2x) + optional SumP trees; Pool computes P-taps and the
two fin ops (stt).  Layout: partition p = 32*quarter + channel; all taps
are free-dim AP offsets on a padded [128, 58, 226] fp16 input.
"""

import os
import sys

for _p in ("/opt/trn_rl_repo", "/opt/pypackages"):
    if _p not in sys.path:
        sys.path.insert(0, _p)

import numpy as np

import concourse.bacc as bacc
import concourse.mybir as mybir
import concourse.tile as tile
from concourse.bass_utils import run_bass_kernel_spmd

B, H, W, C = 8, 224, 224, 32
E = 9
NCORES = 8
Q = 4
RQ = H // Q          # 56
RA = RQ + 2          # 58
WP = W + 2           # 226
P = 128

F32 = mybir.dt.float32
F16 = mybir.dt.float16
F8 = mybir.dt.float8e4

TAPS = [(0, 0)] + [
    (dy, dx) for dy in (-1, 0, 1) for dx in (-1, 0, 1) if not (dy == 0 and dx == 0)
]

# ---- tunables ----
RB = int(os.environ.get("FM_RB", "8"))          # band rows
# taps 0..N_ACT-1: Act produces true A (Abs); rest: DVE 1-op A' = |x - c_e|
# (m_e folded into the SumA matmul weight and Pool's scaled-P stt)
N_ACT = int(os.environ.get("FM_NACT", "7"))
# among Act taps, how many have P on Pool (A'-taps' P is always Pool)
N_PACT = int(os.environ.get("FM_NPACT", "0"))
# number of bands whose SumP runs as DVE tt-tree instead of PE
NP_DVE = int(os.environ.get("FM_NPDVE", "4"))
TREETAIL = bool(int(os.environ.get("FM_TREETAIL", "0")))
# fp8 stacks + DoubleRow paired sum-matmuls for SumA/SumP
FP8 = bool(int(os.environ.get("FM_FP8", "0")))
# walrus rejects gpsimd elementwise ops: route P-taps + fins off Pool
NOPOOL = bool(int(os.environ.get("FM_NOPOOL", "1")))
SDT = F8 if FP8 else F16
S8 = bool(int(os.environ.get("FM_S8", "0")))  # fp8 DoubleRow S-matmuls
SEG = 2                                          # rows per matmul psum slice

_CACHE = {}

# Optionally re-enable walrus LDWEIGHTS dedup (pipeline default disables it;
# our identity-sum matmul runs reuse the same stationary weights 9x).
if bool(int(os.environ.get("FM_LDWOPT", "0"))):
    import concourse.bass_utils as _bu

    _orig_run_command = _bu.run_command

    def _run_command_ldwopt(argv, **kw):
        argv = [
            "--enable-ldw-opt=true" if a == "--enable-ldw-opt=false" else a
            for a in argv
        ]
        return _orig_run_command(argv, **kw)

    _bu.run_command = _run_command_ldwopt


def _build_program(num_devices=NCORES):
    nc = bacc.Bacc(
        "TRN2", target_bir_lowering=False, debug=False, num_devices=num_devices
    )
    x_d = nc.dram_tensor("x", [P, RA, WP], F16, kind="ExternalInput").ap()
    pv_d = nc.dram_tensor("pv", [P, 30], F32, kind="ExternalInput").ap()
    wm_d = nc.dram_tensor("wm", [P, E + 4, P], F16, kind="ExternalInput").ap()
    wa_d = nc.dram_tensor("wa", [P, 12, P], F8, kind="ExternalInput").ap()
    y_d = nc.dram_tensor("y", [P, RQ, W], F16, kind="ExternalOutput").ap()

    with tile.TileContext(nc) as tc:
        _emit(tc, nc, x_d, pv_d, wm_d, wa_d, y_d)
    nc.compile()
    return nc


def _emit(tc, nc, x_d, pv_d, wm_d, wa_d, y_d):
    assert RQ % RB == 0 and RB % SEG == 0
    Abs = mybir.ActivationFunctionType.Abs
    Ident = mybir.ActivationFunctionType.Identity
    add = mybir.AluOpType.add
    sub = mybir.AluOpType.subtract
    mult = mybir.AluOpType.mult
    vmax = mybir.AluOpType.max
    band_ = mybir.AluOpType.bitwise_and
    U16 = mybir.dt.uint16

    sizes = [int(s) for s in os.environ.get("FM_BANDS", "2,4,8,8,8,8,8,8,2").split(",")]
    assert sum(sizes) == RQ and all(s % SEG == 0 for s in sizes)
    starts = [sum(sizes[:i]) for i in range(len(sizes))]
    nbands = len(sizes)
    act_taps = set(range(N_ACT))

    with (
        tc.tile_pool(name="const", bufs=1) as cpool,
        tc.tile_pool(name="xin", bufs=3) as xpool,
        tc.tile_pool(name="stk", bufs=int(os.environ.get("FM_SBUFS", "2"))) as spool,
        tc.tile_pool(name="wrk", bufs=2) as wpool,
        tc.tile_pool(name="acc", bufs=1, space="PSUM") as ppool,
    ):
        pv = cpool.tile([P, 30], F32)
        nc.sync.dma_start(pv[:], pv_d[:])
        wm = cpool.tile([P, E + 4, P], F16)
        nc.sync.dma_start(wm[:], wm_d[:])

        st = {}

        def front(band):
            r0 = starts[band]
            rb = sizes[band]
            xb = xpool.tile([P, rb + 2, WP], F16, tag="xb", bufs=3)
            nc.sync.dma_start(xb[:], x_d[:, r0 : r0 + rb + 2, :])

            def shift(dy, dx, i0, n):
                return xb[
                    :, 1 + dy + i0 : 1 + dy + i0 + n, 1 + dx : 1 + dx + W
                ]

            # S (PE): 9 diag(m_e) matmuls per SEG slice -> m' (Act evict)
            mp = wpool.tile([P, rb, W], F16, tag="mp")
            for s in range(rb // SEG):
                i0 = s * SEG
                sp = ppool.tile([P, SEG * W], F32, tag="sp", bufs=2)
                for e, (dy, dx) in enumerate(TAPS):
                    nc.tensor.matmul(
                        sp[:],
                        wm[:, e, :],
                        shift(dy, dx, i0, SEG),
                        start=(e == 0),
                        stop=(e == E - 1),
                    )
                nc.scalar.activation(
                    mp[:, i0 : i0 + SEG, :],
                    sp[:],
                    Ident,
                    bias=pv[:, 18:19],
                    scale=1.0 / E,
                )

            # A-taps: true A = |m x - k| via Act (1 op) or DVE (2 ts ops)
            A = spool.tile([P, E, rb, W], F16, tag="A")
            for e, (dy, dx) in enumerate(TAPS):
                if e in act_taps:
                    nc.scalar.activation(
                        A[:, e],
                        shift(dy, dx, 0, rb),
                        Abs,
                        bias=pv[:, E + e : E + e + 1],
                        scale=pv[:, e : e + 1],
                    )
                else:
                    tq = wpool.tile([P, rb, W], F16, tag="tq")
                    nc.vector.tensor_scalar(
                        tq[:],
                        shift(dy, dx, 0, rb),
                        pv[:, e : e + 1],
                        pv[:, E + e : E + e + 1],
                        mult,
                        add,
                    )
                    nc.vector.tensor_scalar(
                        A[:, e].bitcast(U16), tq[:].bitcast(U16),
                        0x7FFF, None, band_,
                    )
            st[band] = (mp, A)

        def back(band):
            r0 = starts[band]
            rb = sizes[band]
            mp, A = st.pop(band)

            # P-taps: P_e = max(A_e, m')  (DVE tt)
            Pt = spool.tile([P, E, rb, W], F16, tag="P", bufs=2)
            for e in range(E):
                nc.vector.tensor_tensor(Pt[:, e], A[:, e], mp[:], vmax)

            # n1 = sum A (PE identity) -> u2 = (n1+9)/9 (Act evict)
            u2 = wpool.tile([P, rb, W], F16, tag="u2")
            for s in range(rb // SEG):
                i0 = s * SEG
                np_ = ppool.tile([P, SEG * W], F32, tag="np", bufs=3)
                for e in range(E):
                    nc.tensor.matmul(
                        np_[:],
                        wm[:, E, :],
                        A[:, e, i0 : i0 + SEG, :],
                        start=(e == 0),
                        stop=(e == E - 1),
                    )
                nc.scalar.activation(
                    u2[:, i0 : i0 + SEG, :],
                    np_[:],
                    Ident,
                    bias=1.0,
                    scale=1.0 / E,
                )

            # Pp = (2/9)(sum P) - m'  -> q = Pp - u2 = (var-9)/9
            q_t = wpool.tile([P, rb, W], F16, tag="q")
            if (band < nbands - NP_DVE) if TREETAIL else (band >= NP_DVE):
                for s in range(rb // SEG):
                    i0 = s * SEG
                    pp = ppool.tile([P, SEG * W], F32, tag="pp", bufs=3)
                    for e in range(E):
                        nc.tensor.matmul(
                            pp[:],
                            wm[:, E + 2, :],
                            Pt[:, e, i0 : i0 + SEG, :],
                            start=(e == 0),
                            stop=False,
                        )
                    nc.tensor.matmul(
                        pp[:],
                        wm[:, E + 1, :],
                        mp[:, i0 : i0 + SEG, :],
                        start=False,
                        stop=True,
                    )
                    nc.vector.tensor_tensor(
                        q_t[:, i0 : i0 + SEG, :],
                        pp[:],
                        u2[:, i0 : i0 + SEG, :],
                        sub,
                    )
            else:
                acc = wpool.tile([P, rb, W], F16, tag="acc")
                acc2 = wpool.tile([P, rb, W], F16, tag="acc2")
                nc.vector.tensor_scalar(acc[:], mp[:], -4.5, None, mult)
                cur, other = acc, acc2
                for e in range(E):
                    nc.vector.tensor_tensor(other[:], cur[:], Pt[:, e], add)
                    cur, other = other, cur
                nc.vector.tensor_scalar(cur[:], cur[:], 2.0 / E, None, mult)
                nc.vector.tensor_tensor(q_t[:], cur[:], u2[:], sub)

            # u0 = u2 - 2 ; y = u0 * q
            u0 = wpool.tile([P, rb, W], F16, tag="u0")
            nc.vector.tensor_scalar(u0[:], u2[:], 2.0, None, sub)
            yb = wpool.tile([P, rb, W], F16, tag="yb")
            nc.vector.tensor_tensor(yb[:], u0[:], q_t[:], mult)
            nc.sync.dma_start(y_d[:, r0 : r0 + rb, :], yb[:])

        depth = int(os.environ.get("FM_DEPTH", "1"))
        for band in range(nbands + depth):
            if band < nbands:
                front(band)
            if band >= depth:
                back(band - depth)


def _host_pack(inp, kern, mask):
    inp = np.asarray(inp, dtype=np.float32)
    kern = np.asarray(kern, dtype=np.float32).reshape(E, C)
    mask = np.asarray(mask, dtype=np.float32).reshape(E, C)

    m = np.abs(mask) / (np.abs(mask).max() + np.float32(1e-6))  # [E,C]
    kbar = kern.mean(axis=0)

    cidx = np.arange(P) % C
    pv = np.zeros((P, 30), np.float32)
    for e in range(E):
        pv[:, e] = m[e][cidx]
        pv[:, E + e] = -kern[e][cidx]
        pv[:, 20 + e] = (kern[e] / m[e])[cidx]
    pv[:, 18] = -kbar[cidx]
    pv[:, 19] = -1.0
    pv[:, 29] = -2.0

    wm = np.zeros((P, E + 4, P), np.float16)
    rng = np.arange(P)
    for e in range(E):
        wm[rng, e, rng] = m[e][cidx]
    wm[rng, E, rng] = 1.0
    wm[rng, E + 1, rng] = -1.0          # fold: -1 * m'  (scaled SumP)
    wm[rng, E + 2, rng] = 2.0 / E       # (2/9)*I for SumP
    wm[rng, E + 3, rng] = -2.0          # -2 * ones

    import ml_dtypes

    wa = np.zeros((P, 12, P), ml_dtypes.float8_e4m3)
    for j, (a, b) in enumerate(((1, 3), (4, 5), (6, 8))):
        wa[rng, 2 * j, rng] = m[a][cidx]
        wa[rng, 2 * j + 1, rng] = m[b][cidx]

    in_maps = []
    for b in range(B):
        padded = np.pad(inp[b], ((1, 1), (1, 1), (0, 0)))  # [226,226,32]
        qs = np.stack(
            [padded[RQ * q : RQ * q + RA] for q in range(Q)], axis=0
        )
        x_dev = np.ascontiguousarray(
            qs.transpose(0, 3, 1, 2).reshape(P, RA, WP).astype(np.float16)
        )
        in_maps.append({"x": x_dev, "pv": pv, "wm": wm, "wa": wa})
    return in_maps


def _host_unpack(results):
    out = np.empty((B, H, W, C), np.float32)
    for b in range(B):
        y = np.asarray(results[b]["y"], dtype=np.float32).reshape(Q, C, RQ, W)
        out[b] = y.transpose(0, 2, 3, 1).reshape(H, W, C)
    return out


LAST_PROFILE = {}


def _install_ntff_shim():
    import contextlib
    import ctypes
    import types

    if "antenv.axon_hooks" in sys.modules:
        return
    so_path = "/opt/axon/libaxon_pjrt.so"
    try:
        lib = ctypes.CDLL(so_path)
    except OSError:
        return
    if not hasattr(lib, "axon_start_nrt_profile"):
        return
    lib.axon_start_nrt_profile.argtypes = [
        ctypes.POINTER(ctypes.c_int64),
        ctypes.c_size_t,
    ]
    lib.axon_start_nrt_profile.restype = ctypes.c_int64
    lib.axon_stop_nrt_profile.argtypes = [ctypes.c_char_p]
    lib.axon_stop_nrt_profile.restype = ctypes.c_int64

    @contextlib.contextmanager
    def _hook(output_dir, device_ids):
        import jax

        jax.devices()
        if device_ids:
            ids = (ctypes.c_int64 * len(device_ids))(*device_ids)
            rc = lib.axon_start_nrt_profile(ids, len(device_ids))
        else:
            rc = lib.axon_start_nrt_profile(None, 0)
        if rc != 0:
            raise RuntimeError(f"axon_start_nrt_profile rc={rc}")
        try:
            yield
        finally:
            n = lib.axon_stop_nrt_profile(str(output_dir).encode())
            if n < 0:
                raise RuntimeError(f"axon_stop_nrt_profile rc={n}")
            print(f"ntff profile: {n} file(s) written to {output_dir}")

    mod = types.ModuleType("antenv.axon_hooks")
    mod._hook = _hook
    mod.get_axon_ntff_profile_hook = lambda: mod._hook
    mod.set_axon_ntff_profile_hook = lambda h: setattr(mod, "_hook", h)
    sys.modules["antenv.axon_hooks"] = mod


def kernel(inp, kernel, mask):
    if "nc" not in _CACHE:
        _CACHE["nc"] = _build_program()
    nc = _CACHE["nc"]

    in_maps = _host_pack(inp, kernel, mask)
    trace = bool(int(os.environ.get("FM_TRACE", "0")))
    if trace:
        _install_ntff_shim()
    res = run_bass_kernel_spmd(
        nc, in_maps, core_ids=list(range(NCORES)), trace=trace
    )
    LAST_PROFILE["exec_time_ns"] = res.exec_time_ns
    LAST_PROFILE["mean_exec_time_ns"] = res.mean_exec_time_ns
    return _host_unpack(res.results)
